# revision 51
# baseline (speedup 1.0000x reference)
"""LLaMA causal self-attention (GQA) on 8 Trainium2 NeuronCores.

Sharding: 2-way data-parallel over batch x 4-way tensor-parallel over KV
groups. Core cid handles batch b=cid//4 and KV group g=cid%4 (q heads
4g..4g+3, kv head g). Each core computes a partial output y_partial =
att_heads @ Wo_rows; the host sums the 4 partials per batch and adds bo.

Per-core pipeline (layouts chosen so matmul contraction is on the
partition dim and every matmul streams >=512 columns):
  P1: QKV projection (x^T chunks stationary, W moving), RMSNorm (ACT
      Square+accum -> Rsqrt), RoPE (DVE/GPSIMD, norm weights folded into
      trig tables host-side), PE-transpose q/k to [d, t]; transposes are
      emitted one tile late so the PE never waits on the RMSNorm chain.
  P2: attention transposed + q-block-major: for each 128-col q block,
      scoresT[k, 4*128 qcols] = kT_j^T @ [q0..q3] in ONE 512-wide matmul
      (GQA: all 4 q heads share kv head), causal mask added via an
      identity-stationary matmul on the diagonal block, exp without max
      subtraction (RMS-normed scores are bounded), PV and denominator
      (all-ones stationary) accumulate in PSUM over j.  The j loop is
      software-pipelined two blocks deep so PE never waits for exp.
  P3: output projection from attT chunks, PSUM -> SBUF -> DRAM.
"""

import os
from contextlib import ExitStack

import numpy as np

B, T, C = 2, 2048, 2048
H, KV = 16, 4
D = 128
HQ = H // KV        # q heads per core = 4
TB = T // 128       # 16
CB = C // 128       # 16
EPS = 1e-5
SCALE = float(np.float32(1.0) / np.sqrt(np.float32(D)))

_CACHE = {}


def _build(dt_name, dtx_name, has_bias):
    import concourse.bass as bass
    import concourse.bacc as bacc
    from concourse import mybir
    from concourse.tile import TileContext

    DT = getattr(mybir.dt, dt_name)
    DTX = getattr(mybir.dt, dtx_name)   # x / Wqkv (QKV matmul operands)
    F32 = mybir.dt.float32
    AF = mybir.ActivationFunctionType
    ALU = mybir.AluOpType

    nc = bacc.Bacc(None, target_bir_lowering=False)
    xt = nc.dram_tensor("xt", [TB, 128, CB * 128], DTX, kind="ExternalInput")
    wqkv = nc.dram_tensor("wqkv", [4, 128, 4 * 768], DTX, kind="ExternalInput")
    bqkv = nc.dram_tensor("bqkv", [1, 768], DT, kind="ExternalInput")
    trig = nc.dram_tensor("trig", [TB, 128, 8 * 64], DT, kind="ExternalInput")
    maskt4 = nc.dram_tensor("maskt4", [128, 512], DT, kind="ExternalInput")
    cst = nc.dram_tensor("cst", [2, 128, 128], DT, kind="ExternalInput")
    wo = nc.dram_tensor("wo", [HQ, 128, C], DT, kind="ExternalInput")
    BF16 = mybir.dt.bfloat16
    y = nc.dram_tensor("y", [T, C], BF16, kind="ExternalOutput")

    def bcast(apv, n, inner):
        # broadcast an AP along a new middle dim of size n
        return bass.AP(tensor=apv.tensor, offset=apv.offset,
                       ap=[list(apv.ap[0]), [0, n], [1, inner]])

    with TileContext(nc) as tc, ExitStack() as ctx:
        persist = ctx.enter_context(tc.tile_pool(name="persist", bufs=1))
        ones = persist.tile([128, 128], DT)
        ident = persist.tile([128, 128], DT)
        nc.sync.dma_start(out=ones, in_=cst[0])
        nc.sync.dma_start(out=ident, in_=cst[1])
        maskt_sb = persist.tile([128, 512], DT)
        nc.sync.dma_start(out=maskt_sb, in_=maskt4[:, :])
        eps_ap = persist.tile([128, 1], F32)
        nc.gpsimd.memset(eps_ap, EPS)
        if has_bias:
            bq_sb = persist.tile([1, 768], DT)
            nc.sync.dma_start(out=bq_sb, in_=bqkv[:, :])
        # [d, qblock, seg*128]: segs 0..3 = q heads, seg 4 = k
        qkT = persist.tile([128, TB, 640], DT)
        vbuf = persist.tile([128, TB, 128], DT)   # [t-in-block, j, d]

        # p1q (qr tiles) and p1tp (transpose PSUM, 1 bank in bf16) span
        # P1 AND P2 so the last tile's transposes can be emitted inside
        # P2, hiding the final RMSNorm/RoPE chain under P2 matmuls.
        p1q = ctx.enter_context(tc.tile_pool(name="p1q", bufs=3))
        p1tp = ctx.enter_context(
            tc.tile_pool(name="p1tp", bufs=1, space="PSUM"))
        # P2/P3 SBUF pools are hoisted before the P1 pools so their
        # addresses do not alias freed P1 tiles (aliasing would chain
        # P2's first writes behind P1's last readers), and so the wo
        # prefetch DMA can start immediately.
        p23 = ctx.enter_context(tc.tile_pool(name="p23", bufs=1))
        attT = p23.tile([128, TB, 512], DT)     # [d, qblock, (h,q)]
        wo_sb = p23.tile([128, HQ, C], DT)
        p2p = ctx.enter_context(tc.tile_pool(name="p2p", bufs=3))
        p2n = ctx.enter_context(tc.tile_pool(name="p2n", bufs=2))
        p3 = ctx.enter_context(tc.tile_pool(name="p3", bufs=2))

        def emit_transpose(qr_t, tt_t):
            tp = p1tp.tile([128, 640], DT, tag="tp")
            for s in range(5):
                nc.tensor.transpose(tp[:, s * 128:(s + 1) * 128],
                                    qr_t[:, s * 128:(s + 1) * 128], ident)
            nc.scalar.activation(out=qkT[:, tt_t, :], in_=tp,
                                 func=AF.Copy, scale=1.0, bias=0.0)

        # ---------------- P1: QKV + RMSNorm + RoPE + transpose ----------
        with tc.tile_pool(name="p1", bufs=3) as p1, \
             tc.tile_pool(name="p1x", bufs=3) as p1x, \
             tc.tile_pool(name="p1s", bufs=3) as p1s, \
             tc.tile_pool(name="p1w", bufs=1) as p1w, \
             tc.tile_pool(name="p1ps", bufs=2, space="PSUM") as p1ps:
            xt_tiles = {}

            def load_xt(tt):
                xtall = p1x.tile([128, CB, 128], DTX, tag="xt")
                nc.sync.dma_start(out=xtall, in_=xt[tt])
                trig_sb = p1x.tile([128, 8, 64], DT, tag="trig")
                nc.sync.dma_start(out=trig_sb, in_=trig[tt])
                xt_tiles[tt] = (xtall, trig_sb)

            # Process tile 15 FIRST: the last-finishing RMSNorm/RoPE
            # chains then belong to tiles 13/14, which P2 only consumes
            # at qb=13/14 -- their transposes are deferred into P2 and
            # never stall the PE.
            tts = [TB - 1] + list(range(TB - 1))

            # first x tile ahead of the weights so the PE can start early;
            # one tile per cc chunk so each matmul waits only on its own
            # chunk's DMA (tile-granular dependency tracking)
            load_xt(tts[0])
            wq_groups = []
            for gg in range(4):
                wt = p1w.tile([128, 4, 768], DTX, tag=f"w{gg}")
                nc.sync.dma_start(out=wt, in_=wqkv[gg])
                wq_groups.append(wt)
            load_xt(tts[1])
            wq_tiles = [wq_groups[cc // 4][:, cc % 4, :] for cc in range(CB)]

            pend = []  # [(qr tile, tt), ...] waiting for their PE transpose

            for ti, tt in enumerate(tts):
                xtall, trig_sb = xt_tiles.pop(tt)
                if ti + 2 < TB:
                    load_xt(tts[ti + 2])
                if ti == 4:
                    # prefetch Wo mid-P1, after the startup-critical DMAs
                    for h in range(HQ):
                        nc.sync.dma_start(out=wo_sb[:, h, :], in_=wo[h])

                qkv_ps = p1ps.tile([128, 768], F32, tag="qkv")
                for cc in range(CB):
                    nc.tensor.matmul(qkv_ps[:, 0:512], xtall[:, cc, :],
                                     wq_tiles[cc][:, 0:512],
                                     start=(cc == 0),
                                     stop=(not has_bias and cc == CB - 1))
                    nc.tensor.matmul(qkv_ps[:, 512:768], xtall[:, cc, :],
                                     wq_tiles[cc][:, 512:768],
                                     start=(cc == 0),
                                     stop=(not has_bias and cc == CB - 1))
                if has_bias:
                    nc.tensor.matmul(qkv_ps[:, 0:512], ones[0:1, :],
                                     bq_sb[0:1, 0:512], start=False, stop=True)
                    nc.tensor.matmul(qkv_ps[:, 512:768], ones[0:1, :],
                                     bq_sb[0:1, 512:768], start=False,
                                     stop=True)
                # transposes lag two tiles behind the QKV matmuls so the
                # PE never waits on the RMSNorm/RoPE chain
                if len(pend) >= 2:
                    emit_transpose(*pend.pop(0))

                # RMSNorm stats for 4 q heads + k: one wide Square, one
                # grouped DVE reduce, then rstd = exp(-0.5*ln(ssq/D+EPS)).
                # ln+exp+square+copy live in ONE ACT table set
                # (natural_log_exp_and_others), so the ACT LUT never
                # reloads anywhere in the kernel.
                sq = p1.tile([128, 640], F32, tag="sq")
                nc.scalar.activation(out=sq, in_=qkv_ps[:, 0:640],
                                     func=AF.Square)
                ssq = p1s.tile([128, 8], F32, tag="ssq")
                nc.vector.tensor_reduce(
                    out=ssq[:, 0:5],
                    in_=sq.rearrange("p (s c) -> p s c", c=128),
                    axis=mybir.AxisListType.X, op=ALU.add)
                # rstd = rsqrt(ssq/D + EPS) via DVE-only Newton (linear
                # seed + 4 steps; x is a chi^2_128 mean, concentrated in
                # [0.5, 2], so this is ~1e-4-accurate at the extremes).
                # Keeping Sqrt off ACT means the ACT LUT loads the exp
                # set once and never thrashes.
                x5 = p1s.tile([128, 8], F32, tag="x5")
                nc.scalar.activation(out=x5[:, 0:5], in_=ssq[:, 0:5],
                                     func=AF.Copy, scale=1.0 / D, bias=EPS)
                yy = p1s.tile([128, 8], F32, tag="yy")
                nc.vector.tensor_scalar(out=yy[:, 0:5], in0=x5[:, 0:5],
                                        scalar1=-0.5, scalar2=1.6,
                                        op0=ALU.mult, op1=ALU.add)
                for it in range(4):
                    t1 = p1s.tile([128, 8], F32, tag=f"nt{it}")
                    nc.vector.tensor_mul(t1[:, 0:5], yy[:, 0:5], yy[:, 0:5])
                    t2 = p1s.tile([128, 8], F32, tag=f"nu{it}")
                    nc.vector.scalar_tensor_tensor(
                        out=t2[:, 0:5], in0=t1[:, 0:5], scalar=-0.5,
                        in1=x5[:, 0:5], op0=ALU.mult, op1=ALU.mult)
                    yn = p1s.tile([128, 8], F32, tag=f"ny{it}")
                    nc.vector.scalar_tensor_tensor(
                        out=yn[:, 0:5], in0=t2[:, 0:5], scalar=1.5,
                        in1=yy[:, 0:5], op0=ALU.add, op1=ALU.mult)
                    yy = yn
                rstd = yy

                # normalize q/k in one DVE op (rstd bcast along free dim)
                qn = p1.tile([128, 640], F32, tag="qn")
                rstd_b = bass.AP(tensor=rstd.tensor, offset=rstd.offset,
                                 ap=[list(rstd.ap[0]), [1, 5], [0, 128]])
                nc.vector.tensor_mul(qn.rearrange("p (s c) -> p s c", c=128),
                                     qkv_ps[:, 0:640].rearrange(
                                         "p (s c) -> p s c", c=128),
                                     rstd_b)
                nc.scalar.activation(out=vbuf[:, tt, :], in_=qkv_ps[:, 640:768],
                                     func=AF.Copy, scale=1.0, bias=0.0)

                # RoPE (norm weights folded into trig tables host-side)
                qr = p1q.tile([128, 640], DT, tag="qr")

                def rope(seg0, nseg, toff):
                    src = qn[:, seg0 * 128:(seg0 + nseg) * 128]
                    dst = qr[:, seg0 * 128:(seg0 + nseg) * 128]
                    sev = src.rearrange("p (h j t) -> p h t j", t=2, j=64)
                    dev = dst.rearrange("p (h j t) -> p h t j", t=2, j=64)
                    qe, qo = sev[:, :, 0, :], sev[:, :, 1, :]
                    re, ro = dev[:, :, 0, :], dev[:, :, 1, :]
                    ce = bcast(trig_sb[:, toff + 0, :], nseg, 64)
                    so = bcast(trig_sb[:, toff + 1, :], nseg, 64)
                    se = bcast(trig_sb[:, toff + 2, :], nseg, 64)
                    co = bcast(trig_sb[:, toff + 3, :], nseg, 64)
                    ta = p1s.tile([128, nseg, 64], F32, tag=f"ra{toff}")
                    tb = p1s.tile([128, nseg, 64], F32, tag=f"rb{toff}")
                    nc.vector.tensor_mul(ta, qe, ce)
                    nc.gpsimd.tensor_mul(tb, qo, so)
                    nc.vector.tensor_sub(re, ta, tb)
                    tc_ = p1s.tile([128, nseg, 64], F32, tag=f"rc{toff}")
                    td = p1s.tile([128, nseg, 64], F32, tag=f"rd{toff}")
                    nc.gpsimd.tensor_mul(tc_, qe, se)
                    nc.vector.tensor_mul(td, qo, co)
                    nc.gpsimd.tensor_add(ro, tc_, td)

                rope(0, 4, 0)   # q heads, tables 0..3
                rope(4, 1, 4)   # k, tables 4..7

                pend.append((qr, tt))

            # transposes for the last two processed tiles (13, 14) are
            # deferred into P2 where their chains have huge slack

        # ------- P2: attention (q-block-major, 512-wide, pipelined) -----
        # ------- P3: output projection -----------------------------------
        if True:
            with tc.tile_pool(name="p2acc", bufs=2, space="PSUM") as p2acc, \
                 tc.tile_pool(name="p2sc", bufs=3, space="PSUM") as p2sc:
                for qb in range(TB):
                    if qb in (6, 7) and pend:
                        emit_transpose(*pend.pop(0))
                    outT = p2acc.tile([128, 512], F32, tag="outT")
                    sums = p2acc.tile([128, 512], F32, tag="sums")
                    sc_tiles = {}

                    def emit_sc(j, qb=qb, sc_tiles=sc_tiles):
                        sc = p2sc.tile([128, 512], F32, tag="sc")
                        nc.tensor.matmul(sc, qkT[:, j, 512:640],
                                         qkT[:, qb, 0:512],
                                         start=True, stop=(j != qb))
                        if j == qb:
                            nc.tensor.matmul(sc, ident, maskt_sb,
                                             start=False, stop=True)
                        sc_tiles[j] = sc

                    for j0 in range(min(2, qb + 1)):
                        emit_sc(j0)
                    for j in range(qb + 1):
                        pT = p2p.tile([128, 512], DT, tag="pT")
                        nc.scalar.activation(out=pT, in_=sc_tiles.pop(j),
                                             func=AF.Exp, scale=SCALE)
                        if j + 2 <= qb:
                            emit_sc(j + 2)
                        nc.tensor.matmul(outT, vbuf[:, j, :], pT,
                                         start=(j == 0), stop=(j == qb),
                                         skip_group_check=True)
                        nc.tensor.matmul(sums, ones, pT,
                                         start=(j == 0), stop=(j == qb),
                                         skip_group_check=True)

                    inv = p2n.tile([128, 512], F32, tag="inv")
                    nc.vector.reciprocal_approx_fast(out=inv, in_=sums)
                    nc.vector.tensor_mul(attT[:, qb, :], outT, inv)

            # ---------------- P3: output projection ----------------------
            # c4-outer with one PSUM tile per 512-col chunk: the chunk's
            # PSUM->SBUF copy overlaps the next chunk's matmuls
            with tc.tile_pool(name="p3ps", bufs=2, space="PSUM") as p3ps:
                for tt in range(TB):
                    for c4 in range(4):
                        y_ps = p3ps.tile([128, 512], F32, tag="y")
                        for h in range(HQ):
                            nc.tensor.matmul(
                                y_ps,
                                attT[:, tt, h * 128:(h + 1) * 128],
                                wo_sb[:, h, c4 * 512:(c4 + 1) * 512],
                                start=(h == 0), stop=(h == HQ - 1))
                        y_sb = p3.tile([128, 512], BF16, tag=f"ysb{c4 % 2}")
                        if c4 % 2 == 0:
                            nc.scalar.activation(out=y_sb, in_=y_ps,
                                                 func=AF.Copy, scale=1.0,
                                                 bias=0.0)
                        else:
                            nc.vector.tensor_copy(y_sb, y_ps)
                        nc.sync.dma_start(
                            out=y[tt * 128:(tt + 1) * 128,
                                  c4 * 512:(c4 + 1) * 512],
                            in_=y_sb)

    nc.compile()
    return nc


def _prep_core_inputs(b, g, x, Wq, bq, Wk, bk, Wv, bv, Wo, bo, qn_w, kn_w,
                      freqs_cos, freqs_sin, mask, np_dtx=np.float32,
                      np_dt=np.float32):
    f32 = np.float32
    xb = np.ascontiguousarray(x[b], dtype=f32)
    # [tt, csub, cc, tcol]: xt[tt][p][cc*128+tc] = x[b][tt*128+tc][cc*128+p]
    xt = np.ascontiguousarray(
        xb.reshape(TB, 128, CB, 128).transpose(0, 3, 2, 1)
    ).reshape(TB, 128, CB * 128).astype(np_dtx)
    wqkv = np.ascontiguousarray(np.concatenate([
        Wq[:, g * 512:(g + 1) * 512],
        Wk[:, g * 128:(g + 1) * 128],
        Wv[:, g * 128:(g + 1) * 128],
    ], axis=1).reshape(4, 4, 128, 768).transpose(0, 2, 1, 3)
    ).reshape(4, 128, 4 * 768).astype(np_dtx)
    bqkv = np.concatenate([
        bq[g * 512:(g + 1) * 512], bk[g * 128:(g + 1) * 128],
        bv[g * 128:(g + 1) * 128],
    ]).reshape(1, 768).astype(np_dt)
    cos = freqs_cos.astype(f32)
    sin = freqs_sin.astype(f32)
    qe, qo = qn_w[0::2].astype(f32), qn_w[1::2].astype(f32)
    ke, ko = kn_w[0::2].astype(f32), kn_w[1::2].astype(f32)
    # tables: [ce, so, se, co] for q then for k; layout [TB, 128, 8*64]
    tabs = np.stack([cos * qe, sin * qo, sin * qe, cos * qo,
                     cos * ke, sin * ko, sin * ke, cos * ko], axis=1)  # [T, 8, 64]
    trig = np.ascontiguousarray(tabs.reshape(TB, 128, 8 * 64),
                                dtype=f32).astype(np_dt)
    maskt = np.ascontiguousarray(mask[0, 0, :128, :128].T, dtype=f32)
    maskt4 = np.ascontiguousarray(np.tile(maskt, (1, HQ)),
                                  dtype=f32).astype(np_dt)
    cst = np.stack([np.ones((128, 128), f32),
                    np.eye(128, dtype=f32)]).astype(np_dt)
    wo_t = np.ascontiguousarray(
        Wo[g * 512:(g + 1) * 512].reshape(HQ, 128, C),
        dtype=f32).astype(np_dt)
    return {"xt": xt, "wqkv": wqkv, "bqkv": bqkv, "trig": trig,
            "maskt4": maskt4, "cst": cst, "wo": wo_t}


def kernel(x, Wq, bq, Wk, bk, Wv, bv, Wo, bo, qn_w, kn_w,
           freqs_cos, freqs_sin, mask, _trace=False, _trace_kwargs=None):
    from concourse.bass_utils import run_bass_kernel_spmd

    args = (np.asarray(x), np.asarray(Wq), np.asarray(bq), np.asarray(Wk),
            np.asarray(bk), np.asarray(Wv), np.asarray(bv), np.asarray(Wo),
            np.asarray(bo), np.asarray(qn_w), np.asarray(kn_w),
            np.asarray(freqs_cos), np.asarray(freqs_sin), np.asarray(mask))
    bo_np = args[8].astype(np.float32)
    has_bias = bool(np.any(args[2]) or np.any(args[4]) or np.any(args[6]))

    dt_name = os.environ.get("BASS_ATTN_DT", "bfloat16")
    dtx_name = os.environ.get("BASS_ATTN_DTX", "bfloat16")
    key = (dt_name, dtx_name, has_bias)
    if key not in _CACHE:
        _CACHE[key] = _build(dt_name, dtx_name, has_bias)
    nc = _CACHE[key]

    import ml_dtypes
    np_dtx = ml_dtypes.bfloat16 if dtx_name == "bfloat16" else np.float32
    np_dt = ml_dtypes.bfloat16 if dt_name == "bfloat16" else np.float32
    in_maps = [_prep_core_inputs(cid // 4, cid % 4, *args, np_dtx=np_dtx,
                                 np_dt=np_dt)
               for cid in range(8)]
    res = run_bass_kernel_spmd(nc, in_maps, core_ids=list(range(8)),
                               trace=_trace, **(_trace_kwargs or {}))
    outs = [np.asarray(res.results[i]["y"]).astype(np.float32)
            for i in range(8)]
    yfull = np.empty((B, T, C), dtype=np.float32)
    for b in range(B):
        yfull[b] = outs[4 * b] + outs[4 * b + 1] + outs[4 * b + 2] + outs[4 * b + 3]
        yfull[b] += bo_np[None, :]
    if _trace:
        kernel._last_result = res
    return yfull


# revision 52
# speedup vs baseline: 1.1404x; 1.1404x over previous
"""LLaMA causal self-attention (GQA) on 8 Trainium2 NeuronCores.

Sharding: 2-way data-parallel over batch x 4-way tensor-parallel over KV
groups. Core cid handles batch b=cid//4 and KV group g=cid%4 (q heads
4g..4g+3, kv head g). Each core computes a partial output y_partial =
att_heads @ Wo_rows; the host sums the 4 partials per batch and adds bo.

Per-core pipeline (layouts chosen so matmul contraction is on the
partition dim and every matmul streams >=512 columns):
  P1: QKV projection (x^T chunks stationary, W moving), RMSNorm (ACT
      Square+accum -> Rsqrt), RoPE (DVE/GPSIMD, norm weights folded into
      trig tables host-side), PE-transpose q/k to [d, t]; transposes are
      emitted one tile late so the PE never waits on the RMSNorm chain.
  P2: attention transposed + q-block-major: for each 128-col q block,
      scoresT[k, 4*128 qcols] = kT_j^T @ [q0..q3] in ONE 512-wide matmul
      (GQA: all 4 q heads share kv head), causal mask added via an
      identity-stationary matmul on the diagonal block, exp without max
      subtraction (RMS-normed scores are bounded), PV and denominator
      (all-ones stationary) accumulate in PSUM over j.  The j loop is
      software-pipelined two blocks deep so PE never waits for exp.
  P3: output projection from attT chunks, PSUM -> SBUF -> DRAM.
"""

import os
from contextlib import ExitStack

import numpy as np

B, T, C = 2, 2048, 2048
H, KV = 16, 4
D = 128
HQ = H // KV        # q heads per core = 4
TB = T // 128       # 16
CB = C // 128       # 16
EPS = 1e-5
SCALE = float(np.float32(1.0) / np.sqrt(np.float32(D)))

_CACHE = {}


def _build(dt_name, dtx_name, has_bias):
    import concourse.bass as bass
    import concourse.bacc as bacc
    from concourse import mybir
    from concourse.tile import TileContext

    DT = getattr(mybir.dt, dt_name)
    DTX = getattr(mybir.dt, dtx_name)   # x / Wqkv (QKV matmul operands)
    F32 = mybir.dt.float32
    AF = mybir.ActivationFunctionType
    ALU = mybir.AluOpType

    nc = bacc.Bacc(None, target_bir_lowering=False)
    xt = nc.dram_tensor("xt", [TB, 128, CB * 128], DTX, kind="ExternalInput")
    wqkv = nc.dram_tensor("wqkv", [4, 128, 4 * 768], DTX, kind="ExternalInput")
    bqkv = nc.dram_tensor("bqkv", [1, 768], DT, kind="ExternalInput")
    trig = nc.dram_tensor("trig", [TB, 128, 8 * 64], DT, kind="ExternalInput")
    maskt4 = nc.dram_tensor("maskt4", [128, 512], DT, kind="ExternalInput")
    cst = nc.dram_tensor("cst", [2, 128, 128], DT, kind="ExternalInput")
    wo = nc.dram_tensor("wo", [HQ, 128, C], DT, kind="ExternalInput")
    BF16 = mybir.dt.bfloat16
    y = nc.dram_tensor("y", [T, C], BF16, kind="ExternalOutput")

    def bcast(apv, n, inner):
        # broadcast an AP along a new middle dim of size n
        return bass.AP(tensor=apv.tensor, offset=apv.offset,
                       ap=[list(apv.ap[0]), [0, n], [1, inner]])

    with TileContext(nc) as tc, ExitStack() as ctx:
        persist = ctx.enter_context(tc.tile_pool(name="persist", bufs=1))
        ones = persist.tile([128, 128], DT)
        ident = persist.tile([128, 128], DT)
        nc.sync.dma_start(out=ones, in_=cst[0])
        nc.sync.dma_start(out=ident, in_=cst[1])
        maskt_sb = persist.tile([128, 512], DT)
        nc.sync.dma_start(out=maskt_sb, in_=maskt4[:, :])
        eps_ap = persist.tile([128, 1], F32)
        nc.gpsimd.memset(eps_ap, EPS)
        if has_bias:
            bq_sb = persist.tile([1, 768], DT)
            nc.sync.dma_start(out=bq_sb, in_=bqkv[:, :])
        # [d, qblock, seg*128]: segs 0..3 = q heads, seg 4 = k
        qkT = persist.tile([128, TB, 640], DT)
        vbuf = persist.tile([128, TB, 128], DT)   # [t-in-block, j, d]

        # p1q (qr tiles) and p1tp (transpose PSUM, 1 bank in bf16) span
        # P1 AND P2 so the last tile's transposes can be emitted inside
        # P2, hiding the final RMSNorm/RoPE chain under P2 matmuls.
        p1q = ctx.enter_context(tc.tile_pool(name="p1q", bufs=3))
        p1tp = ctx.enter_context(
            tc.tile_pool(name="p1tp", bufs=1, space="PSUM"))
        # P2/P3 SBUF pools are hoisted before the P1 pools so their
        # addresses do not alias freed P1 tiles (aliasing would chain
        # P2's first writes behind P1's last readers), and so the wo
        # prefetch DMA can start immediately.
        p23 = ctx.enter_context(tc.tile_pool(name="p23", bufs=1))
        attT = p23.tile([128, TB, 512], DT)     # [d, qblock, (h,q)]
        wo_sb = p23.tile([128, HQ, C], DT)
        p2p = ctx.enter_context(tc.tile_pool(name="p2p", bufs=3))
        p2n = ctx.enter_context(tc.tile_pool(name="p2n", bufs=2))
        p3 = ctx.enter_context(tc.tile_pool(name="p3", bufs=2))

        def emit_transpose(qr_t, tt_t):
            tp = p1tp.tile([128, 640], DT, tag="tp")
            for s in range(5):
                nc.tensor.transpose(tp[:, s * 128:(s + 1) * 128],
                                    qr_t[:, s * 128:(s + 1) * 128], ident)
            nc.scalar.activation(out=qkT[:, tt_t, :], in_=tp,
                                 func=AF.Copy, scale=1.0, bias=0.0)

        # ---------------- P1: QKV + RMSNorm + RoPE + transpose ----------
        with tc.tile_pool(name="p1", bufs=3) as p1, \
             tc.tile_pool(name="p1x", bufs=3) as p1x, \
             tc.tile_pool(name="p1s", bufs=3) as p1s, \
             tc.tile_pool(name="p1w", bufs=1) as p1w, \
             tc.tile_pool(name="p1ps", bufs=2, space="PSUM") as p1ps:
            xt_tiles = {}

            def load_xt(tt):
                xtall = p1x.tile([128, CB, 128], DTX, tag="xt")
                nc.sync.dma_start(out=xtall, in_=xt[tt])
                trig_sb = p1x.tile([128, 8, 64], DT, tag="trig")
                nc.sync.dma_start(out=trig_sb, in_=trig[tt])
                xt_tiles[tt] = (xtall, trig_sb)

            # Process tile 15 FIRST: the last-finishing RMSNorm/RoPE
            # chains then belong to tiles 13/14, which P2 only consumes
            # at qb=13/14 -- their transposes are deferred into P2 and
            # never stall the PE.
            tts = [TB - 1] + list(range(TB - 1))

            # first x tile ahead of the weights so the PE can start early;
            # one tile per cc chunk so each matmul waits only on its own
            # chunk's DMA (tile-granular dependency tracking)
            load_xt(tts[0])
            wq_groups = []
            for gg in range(4):
                wt = p1w.tile([128, 4, 768], DTX, tag=f"w{gg}")
                nc.sync.dma_start(out=wt, in_=wqkv[gg])
                wq_groups.append(wt)
            load_xt(tts[1])
            wq_tiles = [wq_groups[cc // 4][:, cc % 4, :] for cc in range(CB)]

            pend = []  # [(qr tile, tt), ...] waiting for their PE transpose

            for ti, tt in enumerate(tts):
                xtall, trig_sb = xt_tiles.pop(tt)
                if ti + 2 < TB:
                    load_xt(tts[ti + 2])
                if ti == 4:
                    # prefetch Wo mid-P1, after the startup-critical DMAs
                    for h in range(HQ):
                        nc.sync.dma_start(out=wo_sb[:, h, :], in_=wo[h])

                qkv_ps = p1ps.tile([128, 768], F32, tag="qkv")
                for cc in range(CB):
                    nc.tensor.matmul(qkv_ps[:, 0:512], xtall[:, cc, :],
                                     wq_tiles[cc][:, 0:512],
                                     start=(cc == 0),
                                     stop=(not has_bias and cc == CB - 1))
                    nc.tensor.matmul(qkv_ps[:, 512:768], xtall[:, cc, :],
                                     wq_tiles[cc][:, 512:768],
                                     start=(cc == 0),
                                     stop=(not has_bias and cc == CB - 1))
                if has_bias:
                    nc.tensor.matmul(qkv_ps[:, 0:512], ones[0:1, :],
                                     bq_sb[0:1, 0:512], start=False, stop=True)
                    nc.tensor.matmul(qkv_ps[:, 512:768], ones[0:1, :],
                                     bq_sb[0:1, 512:768], start=False,
                                     stop=True)
                # transposes lag two tiles behind the QKV matmuls so the
                # PE never waits on the RMSNorm/RoPE chain
                if len(pend) >= 2:
                    emit_transpose(*pend.pop(0))

                # RMSNorm stats for 4 q heads + k: one wide Square, one
                # grouped DVE reduce, then rstd = exp(-0.5*ln(ssq/D+EPS)).
                # ln+exp+square+copy live in ONE ACT table set
                # (natural_log_exp_and_others), so the ACT LUT never
                # reloads anywhere in the kernel.
                sq = p1.tile([128, 640], F32, tag="sq")
                nc.scalar.activation(out=sq, in_=qkv_ps[:, 0:640],
                                     func=AF.Square)
                ssq = p1s.tile([128, 8], F32, tag="ssq")
                nc.vector.tensor_reduce(
                    out=ssq[:, 0:5],
                    in_=sq.rearrange("p (s c) -> p s c", c=128),
                    axis=mybir.AxisListType.X, op=ALU.add)
                sq5 = p1s.tile([128, 8], F32, tag="sq5")
                nc.scalar.activation(out=sq5[:, 0:5], in_=ssq[:, 0:5],
                                     func=AF.Sqrt, scale=1.0 / D, bias=eps_ap)
                rstd = p1s.tile([128, 8], F32, tag="rstd")
                nc.vector.reciprocal(out=rstd[:, 0:5], in_=sq5[:, 0:5])

                # normalize q/k in one DVE op (rstd bcast along free dim)
                qn = p1.tile([128, 640], F32, tag="qn")
                rstd_b = bass.AP(tensor=rstd.tensor, offset=rstd.offset,
                                 ap=[list(rstd.ap[0]), [1, 5], [0, 128]])
                nc.vector.tensor_mul(qn.rearrange("p (s c) -> p s c", c=128),
                                     qkv_ps[:, 0:640].rearrange(
                                         "p (s c) -> p s c", c=128),
                                     rstd_b)
                nc.scalar.activation(out=vbuf[:, tt, :], in_=qkv_ps[:, 640:768],
                                     func=AF.Copy, scale=1.0, bias=0.0)

                # RoPE (norm weights folded into trig tables host-side)
                qr = p1q.tile([128, 640], DT, tag="qr")

                def rope(seg0, nseg, toff):
                    src = qn[:, seg0 * 128:(seg0 + nseg) * 128]
                    dst = qr[:, seg0 * 128:(seg0 + nseg) * 128]
                    sev = src.rearrange("p (h j t) -> p h t j", t=2, j=64)
                    dev = dst.rearrange("p (h j t) -> p h t j", t=2, j=64)
                    qe, qo = sev[:, :, 0, :], sev[:, :, 1, :]
                    re, ro = dev[:, :, 0, :], dev[:, :, 1, :]
                    ce = bcast(trig_sb[:, toff + 0, :], nseg, 64)
                    so = bcast(trig_sb[:, toff + 1, :], nseg, 64)
                    se = bcast(trig_sb[:, toff + 2, :], nseg, 64)
                    co = bcast(trig_sb[:, toff + 3, :], nseg, 64)
                    ta = p1s.tile([128, nseg, 64], F32, tag=f"ra{toff}")
                    tb = p1s.tile([128, nseg, 64], F32, tag=f"rb{toff}")
                    nc.vector.tensor_mul(ta, qe, ce)
                    nc.gpsimd.tensor_mul(tb, qo, so)
                    nc.vector.tensor_sub(re, ta, tb)
                    tc_ = p1s.tile([128, nseg, 64], F32, tag=f"rc{toff}")
                    td = p1s.tile([128, nseg, 64], F32, tag=f"rd{toff}")
                    nc.gpsimd.tensor_mul(tc_, qe, se)
                    nc.vector.tensor_mul(td, qo, co)
                    nc.gpsimd.tensor_add(ro, tc_, td)

                rope(0, 4, 0)   # q heads, tables 0..3
                rope(4, 1, 4)   # k, tables 4..7

                pend.append((qr, tt))

            # transposes for the last two processed tiles (13, 14) are
            # deferred into P2 where their chains have huge slack

        # ------- P2: attention (q-block-major, 512-wide, pipelined) -----
        # ------- P3: output projection -----------------------------------
        if True:
            with tc.tile_pool(name="p2acc", bufs=2, space="PSUM") as p2acc, \
                 tc.tile_pool(name="p2sc", bufs=3, space="PSUM") as p2sc:
                for qb in range(TB):
                    if qb in (6, 7) and pend:
                        emit_transpose(*pend.pop(0))
                    outT = p2acc.tile([128, 512], F32, tag="outT")
                    sums = p2acc.tile([128, 512], F32, tag="sums")
                    sc_tiles = {}

                    def emit_sc(j, qb=qb, sc_tiles=sc_tiles):
                        sc = p2sc.tile([128, 512], F32, tag="sc")
                        nc.tensor.matmul(sc, qkT[:, j, 512:640],
                                         qkT[:, qb, 0:512],
                                         start=True, stop=(j != qb))
                        if j == qb:
                            nc.tensor.matmul(sc, ident, maskt_sb,
                                             start=False, stop=True)
                        sc_tiles[j] = sc

                    for j0 in range(min(2, qb + 1)):
                        emit_sc(j0)
                    for j in range(qb + 1):
                        pT = p2p.tile([128, 512], DT, tag="pT")
                        nc.scalar.activation(out=pT, in_=sc_tiles.pop(j),
                                             func=AF.Exp, scale=SCALE)
                        if j + 2 <= qb:
                            emit_sc(j + 2)
                        nc.tensor.matmul(outT, vbuf[:, j, :], pT,
                                         start=(j == 0), stop=(j == qb),
                                         skip_group_check=True)
                        nc.tensor.matmul(sums, ones, pT,
                                         start=(j == 0), stop=(j == qb),
                                         skip_group_check=True)

                    inv = p2n.tile([128, 512], F32, tag="inv")
                    nc.vector.reciprocal_approx_fast(out=inv, in_=sums)
                    nc.vector.tensor_mul(attT[:, qb, :], outT, inv)

            # ---------------- P3: output projection ----------------------
            # c4-outer with one PSUM tile per 512-col chunk: the chunk's
            # PSUM->SBUF copy overlaps the next chunk's matmuls
            with tc.tile_pool(name="p3ps", bufs=2, space="PSUM") as p3ps:
                for tt in range(TB):
                    for c4 in range(4):
                        y_ps = p3ps.tile([128, 512], F32, tag="y")
                        for h in range(HQ):
                            nc.tensor.matmul(
                                y_ps,
                                attT[:, tt, h * 128:(h + 1) * 128],
                                wo_sb[:, h, c4 * 512:(c4 + 1) * 512],
                                start=(h == 0), stop=(h == HQ - 1))
                        y_sb = p3.tile([128, 512], BF16, tag=f"ysb{c4 % 2}")
                        if c4 % 2 == 0:
                            nc.scalar.activation(out=y_sb, in_=y_ps,
                                                 func=AF.Copy, scale=1.0,
                                                 bias=0.0)
                        else:
                            nc.vector.tensor_copy(y_sb, y_ps)
                        nc.sync.dma_start(
                            out=y[tt * 128:(tt + 1) * 128,
                                  c4 * 512:(c4 + 1) * 512],
                            in_=y_sb)

    nc.compile()
    return nc


def _prep_core_inputs(b, g, x, Wq, bq, Wk, bk, Wv, bv, Wo, bo, qn_w, kn_w,
                      freqs_cos, freqs_sin, mask, np_dtx=np.float32,
                      np_dt=np.float32):
    f32 = np.float32
    xb = np.ascontiguousarray(x[b], dtype=f32)
    # [tt, csub, cc, tcol]: xt[tt][p][cc*128+tc] = x[b][tt*128+tc][cc*128+p]
    xt = np.ascontiguousarray(
        xb.reshape(TB, 128, CB, 128).transpose(0, 3, 2, 1)
    ).reshape(TB, 128, CB * 128).astype(np_dtx)
    wqkv = np.ascontiguousarray(np.concatenate([
        Wq[:, g * 512:(g + 1) * 512],
        Wk[:, g * 128:(g + 1) * 128],
        Wv[:, g * 128:(g + 1) * 128],
    ], axis=1).reshape(4, 4, 128, 768).transpose(0, 2, 1, 3)
    ).reshape(4, 128, 4 * 768).astype(np_dtx)
    bqkv = np.concatenate([
        bq[g * 512:(g + 1) * 512], bk[g * 128:(g + 1) * 128],
        bv[g * 128:(g + 1) * 128],
    ]).reshape(1, 768).astype(np_dt)
    cos = freqs_cos.astype(f32)
    sin = freqs_sin.astype(f32)
    qe, qo = qn_w[0::2].astype(f32), qn_w[1::2].astype(f32)
    ke, ko = kn_w[0::2].astype(f32), kn_w[1::2].astype(f32)
    # tables: [ce, so, se, co] for q then for k; layout [TB, 128, 8*64]
    tabs = np.stack([cos * qe, sin * qo, sin * qe, cos * qo,
                     cos * ke, sin * ko, sin * ke, cos * ko], axis=1)  # [T, 8, 64]
    trig = np.ascontiguousarray(tabs.reshape(TB, 128, 8 * 64),
                                dtype=f32).astype(np_dt)
    maskt = np.ascontiguousarray(mask[0, 0, :128, :128].T, dtype=f32)
    maskt4 = np.ascontiguousarray(np.tile(maskt, (1, HQ)),
                                  dtype=f32).astype(np_dt)
    cst = np.stack([np.ones((128, 128), f32),
                    np.eye(128, dtype=f32)]).astype(np_dt)
    wo_t = np.ascontiguousarray(
        Wo[g * 512:(g + 1) * 512].reshape(HQ, 128, C),
        dtype=f32).astype(np_dt)
    return {"xt": xt, "wqkv": wqkv, "bqkv": bqkv, "trig": trig,
            "maskt4": maskt4, "cst": cst, "wo": wo_t}


def kernel(x, Wq, bq, Wk, bk, Wv, bv, Wo, bo, qn_w, kn_w,
           freqs_cos, freqs_sin, mask, _trace=False, _trace_kwargs=None):
    from concourse.bass_utils import run_bass_kernel_spmd

    args = (np.asarray(x), np.asarray(Wq), np.asarray(bq), np.asarray(Wk),
            np.asarray(bk), np.asarray(Wv), np.asarray(bv), np.asarray(Wo),
            np.asarray(bo), np.asarray(qn_w), np.asarray(kn_w),
            np.asarray(freqs_cos), np.asarray(freqs_sin), np.asarray(mask))
    bo_np = args[8].astype(np.float32)
    has_bias = bool(np.any(args[2]) or np.any(args[4]) or np.any(args[6]))

    dt_name = os.environ.get("BASS_ATTN_DT", "bfloat16")
    dtx_name = os.environ.get("BASS_ATTN_DTX", "bfloat16")
    key = (dt_name, dtx_name, has_bias)
    if key not in _CACHE:
        _CACHE[key] = _build(dt_name, dtx_name, has_bias)
    nc = _CACHE[key]

    import ml_dtypes
    np_dtx = ml_dtypes.bfloat16 if dtx_name == "bfloat16" else np.float32
    np_dt = ml_dtypes.bfloat16 if dt_name == "bfloat16" else np.float32
    in_maps = [_prep_core_inputs(cid // 4, cid % 4, *args, np_dtx=np_dtx,
                                 np_dt=np_dt)
               for cid in range(8)]
    res = run_bass_kernel_spmd(nc, in_maps, core_ids=list(range(8)),
                               trace=_trace, **(_trace_kwargs or {}))
    outs = [np.asarray(res.results[i]["y"]).astype(np.float32)
            for i in range(8)]
    yfull = np.empty((B, T, C), dtype=np.float32)
    for b in range(B):
        yfull[b] = outs[4 * b] + outs[4 * b + 1] + outs[4 * b + 2] + outs[4 * b + 3]
        yfull[b] += bo_np[None, :]
    if _trace:
        kernel._last_result = res
    return yfull
